# revision 1
# baseline (speedup 1.0000x reference)
"""TRN2 Bass kernel: relation-weighted scatter-mean GNN aggregation (8-core SPMD).

  out[n] = (1/max(deg(n),1)) * sum_{e: head_e = n} ego[tail_e] * rel[type_e]

Sharding: output entities are split contiguously across the 8 NeuronCores in
512-entity "quads" (4 x 128-entity blocks); the ego table and relation weights
are replicated per core (each core gathers arbitrary tail rows locally), so no
inter-core collective is needed; each core writes its own output slice.

Per core:
  - Edges with head in the core's range are bucketed by head-quad and sorted
    by head.  Consecutive edge pairs (e0,e1) are fused so that ONE indirect-DMA
    descriptor fetches both tail rows: the host builds a paired table
    tbl2[2i]=ego[i], tbl2[2i+1]=ego[sigma(i)] with sigma(tail(e0))=tail(e1)
    (each base tail used at most once per core; leftovers gather singly).
    This halves the dominant cost: gpsimd SWDGE descriptor-prep per 128-row
    indirect DMA call (~1.3us fixed per call on this stack).
  - Per 128-edge tile: rel rows via host-built bf16 one-hot (hi+lo rows, so
    fp32 relation values are exact) x static [2R,D] rel matrix on TensorE;
    msg = gathered_ego * rel_tile on VectorE (fp32); transposed segment matmul
    quad_psum[:, off:off+span] += msg.T @ onehot(head_local) with the narrow
    one-hot built on VectorE from an iota tile (fp32-exact end to end).
  - Per quad: PSUM -> SBUF, PE-transpose each 128 block back to
    [entity, feature], scale by host 1/max(deg,1), DMA out.

All shapes/offsets/schedules are compile-time constants derived from the edge
data; the 8 cores share one SPMD program (cross-core max padding).
"""

import sys
sys.path.insert(0, '/opt/trn_rl_repo')
import numpy as np
import ml_dtypes
from concourse import bass, bacc, mybir
from concourse.masks import make_identity
import concourse.tile as tile

N_CORES = 8
P = 128
QB = 4
QE = QB * P
PAD_LOCAL = -1000.0


def preprocess2(edge_index, edge_type, relation_weight, n_entities, n_rel, d):
    head = np.asarray(edge_index[0], dtype=np.int64)
    tail = np.asarray(edge_index[1], dtype=np.int64)
    etype = np.asarray(edge_type, dtype=np.int64)
    R2 = 2 * n_rel

    n_blocks_total = (n_entities + P - 1) // P
    base_b = n_blocks_total // N_CORES
    rem = n_blocks_total - base_b * N_CORES
    blocks_per_core = [base_b + (1 if k < rem else 0) for k in range(N_CORES)]
    NB = max(blocks_per_core)
    NQ = (NB + QB - 1) // QB
    bstart = np.cumsum([0] + blocks_per_core)
    core_start = bstart[:-1] * P

    counts = np.bincount(head, minlength=n_entities).astype(np.float32)

    # per (core, quad) pairing
    sigma = np.arange(n_entities, dtype=np.int64)[None, :].repeat(N_CORES, 0)
    pairs_cq = [[None] * NQ for _ in range(N_CORES)]   # (e0_idx..) per quad: arrays
    singles_cq = [[None] * NQ for _ in range(N_CORES)]
    for k in range(N_CORES):
        s = core_start[k]
        e_ent = s + blocks_per_core[k] * P
        m = (head >= s) & (head < min(e_ent, n_entities))
        h = head[m] - s
        t = tail[m]
        ty = etype[m]
        o = np.argsort(h, kind='stable')
        h, t, ty = h[o], t[o], ty[o]
        used = np.zeros(n_entities, bool)
        q_of = h // QE
        for q in range(NQ):
            mm = np.where(q_of == q)[0]
            p_base, p_h0, p_t0, p_y0, p_h1, p_t1, p_y1 = [], [], [], [], [], [], []
            s_h, s_t, s_y = [], [], []
            i = 0
            while i + 1 < len(mm):
                a, b = mm[i], mm[i + 1]
                ta, tb = t[a], t[b]
                if not used[ta] and ta != tb:
                    used[ta] = True
                    sigma[k, ta] = tb
                    p_base.append(ta)
                    p_h0.append(h[a] - q * QE); p_t0.append(ta); p_y0.append(ty[a])
                    p_h1.append(h[b] - q * QE); p_t1.append(tb); p_y1.append(ty[b])
                elif not used[tb] and ta != tb:
                    used[tb] = True
                    sigma[k, tb] = ta
                    p_base.append(tb)
                    p_h0.append(h[b] - q * QE); p_t0.append(tb); p_y0.append(ty[b])
                    p_h1.append(h[a] - q * QE); p_t1.append(ta); p_y1.append(ty[a])
                else:
                    s_h.append(h[a] - q * QE); s_t.append(ta); s_y.append(ty[a])
                    s_h.append(h[b] - q * QE); s_t.append(tb); s_y.append(ty[b])
                i += 2
            if i < len(mm):
                a = mm[i]
                s_h.append(h[a] - q * QE); s_t.append(t[a]); s_y.append(ty[a])
            pairs_cq[k][q] = (np.array(p_base, np.int64),
                              np.array(p_h0), np.array(p_t0), np.array(p_y0),
                              np.array(p_h1), np.array(p_t1), np.array(p_y1))
            singles_cq[k][q] = (np.array(s_h), np.array(s_t), np.array(s_y))

    PC = [max(0, max((len(pairs_cq[k][q][0]) + P - 1) // P for k in range(N_CORES)))
          for q in range(NQ)]
    SC = [max((len(singles_cq[k][q][0]) + P - 1) // P for k in range(N_CORES))
          for q in range(NQ)]
    for q in range(NQ):
        if PC[q] == 0 and SC[q] == 0:
            SC[q] = 1
    NCALL = sum(PC) + sum(SC)
    NTT = sum(2 * PC[q] + SC[q] for q in range(NQ))

    idx = np.zeros((N_CORES, P, NCALL), np.int32)
    loc = np.full((N_CORES, P, NTT), PAD_LOCAL, np.float32)
    relhot = np.zeros((N_CORES, NTT, R2, P), ml_dtypes.bfloat16)
    recip = np.zeros((N_CORES, P, NB), np.float32)

    # schedule: list of calls, each with list of tile dicts
    sched = []
    ct = 0
    tt = 0

    def fill_tile(k, tt_, hh, yy, off):
        n = len(hh)
        if n == 0:
            return
        loc[k, :n, tt_] = hh - off
        oh = np.zeros((n_rel, P), np.float32)
        oh[yy, np.arange(n)] = 1.0
        relhot[k, tt_, :n_rel] = oh.astype(ml_dtypes.bfloat16)
        relhot[k, tt_, n_rel:] = oh.astype(ml_dtypes.bfloat16)

    for q in range(NQ):
        first_done = False
        ntiles_q = 2 * PC[q] + SC[q]
        tiles_emitted = 0
        for c in range(PC[q]):
            tiles = []
            for w in range(2):
                lo_u, hi_u = QE, -1
                for k in range(N_CORES):
                    pb, h0, t0, y0, h1, t1, y1 = pairs_cq[k][q]
                    hh = (h0 if w == 0 else h1)[c * P:(c + 1) * P]
                    if len(hh):
                        lo_u = min(lo_u, int(hh.min()))
                        hi_u = max(hi_u, int(hh.max()))
                if not first_done:
                    off, span = 0, QE
                    first_done = True
                elif hi_u < 0:
                    off, span = 0, 1
                else:
                    off, span = lo_u, hi_u - lo_u + 1
                for k in range(N_CORES):
                    pb, h0, t0, y0, h1, t1, y1 = pairs_cq[k][q]
                    hh = (h0 if w == 0 else h1)[c * P:(c + 1) * P]
                    yy = (y0 if w == 0 else y1)[c * P:(c + 1) * P]
                    fill_tile(k, tt, np.asarray(hh), np.asarray(yy), off)
                tiles_emitted += 1
                tiles.append(dict(off=off, span=span,
                                  first=(tiles_emitted == 1),
                                  last=(tiles_emitted == ntiles_q), tt=tt))
                tt += 1
            for k in range(N_CORES):
                pb = pairs_cq[k][q][0][c * P:(c + 1) * P]
                idx[k, :len(pb), ct] = pb
            sched.append(dict(kind='pair', q=q, tiles=tiles, ct=ct))
            ct += 1
        for c in range(SC[q]):
            lo_u, hi_u = QE, -1
            for k in range(N_CORES):
                sh, stl, sy = singles_cq[k][q]
                hh = sh[c * P:(c + 1) * P]
                if len(hh):
                    lo_u = min(lo_u, int(hh.min()))
                    hi_u = max(hi_u, int(hh.max()))
            if not first_done:
                off, span = 0, QE
                first_done = True
            elif hi_u < 0:
                off, span = 0, 1
            else:
                off, span = lo_u, hi_u - lo_u + 1
            for k in range(N_CORES):
                sh, stl, sy = singles_cq[k][q]
                hh = sh[c * P:(c + 1) * P]
                yy = sy[c * P:(c + 1) * P]
                fill_tile(k, tt, np.asarray(hh), np.asarray(yy), off)
                stl_c = stl[c * P:(c + 1) * P]
                idx[k, :len(stl_c), ct] = 2 * stl_c     # row in [2N, d] view
            tiles_emitted += 1
            sched.append(dict(kind='single', q=q, ct=ct,
                              tiles=[dict(off=off, span=span,
                                          first=(tiles_emitted == 1),
                                          last=(tiles_emitted == ntiles_q),
                                          tt=tt)]))
            tt += 1
            ct += 1

    for k in range(N_CORES):
        s = core_start[k]
        for b in range(blocks_per_core[k]):
            ents = s + b * P + np.arange(P)
            valid = ents < n_entities
            c_ = np.where(valid, counts[np.minimum(ents, n_entities - 1)], 0.0)
            recip[k, :, b] = np.where(valid, 1.0 / np.maximum(c_, 1.0), 0.0)

    rw = np.asarray(relation_weight, np.float32)
    rel_hi = rw.astype(ml_dtypes.bfloat16)
    rel_lo = (rw - rel_hi.astype(np.float32)).astype(ml_dtypes.bfloat16)
    rel48 = np.concatenate([rel_hi, rel_lo], axis=0)

    return dict(sched=sched, NQ=NQ, NB=NB, NTT=NTT, NCALL=NCALL, R2=R2, d=d,
                idx=idx, loc=loc, relhot=relhot, recip=recip, rel48=rel48,
                sigma=sigma, blocks_per_core=blocks_per_core,
                core_start=core_start, n_entities=n_entities)


def build_program2(pp, n_table_rows, rep=1):
    d = pp['d']
    R2 = pp['R2']
    NTT, NB, NQ, NCALL = pp['NTT'], pp['NB'], pp['NQ'], pp['NCALL']
    sched = pp['sched']

    nc = bacc.Bacc('TRN2', target_bir_lowering=False, debug=False,
                   num_devices=N_CORES)
    # paired table: row i of [N, 2d] = [ego[i] | ego[sigma(i)]]
    tbl2 = nc.dram_tensor("tbl2", [2 * n_table_rows, d], mybir.dt.float32,
                          kind="ExternalInput").ap()
    tbl2_wide = tbl2.rearrange("(n a) d -> n (a d)", a=2)
    idx_d = nc.dram_tensor("idx", [P, NCALL], mybir.dt.int32,
                           kind="ExternalInput").ap()
    loc_d = nc.dram_tensor("loc", [P, NTT], mybir.dt.float32,
                           kind="ExternalInput").ap()
    relhot_d = nc.dram_tensor("relhot", [NTT, R2, P], mybir.dt.bfloat16,
                              kind="ExternalInput").ap()
    recip_d = nc.dram_tensor("recip", [P, NB], mybir.dt.float32,
                             kind="ExternalInput").ap()
    rel48_d = nc.dram_tensor("rel48", [R2, d], mybir.dt.bfloat16,
                             kind="ExternalInput").ap()
    out_d = nc.dram_tensor("out", [NB * P, d], mybir.dt.float32,
                           kind="ExternalOutput").ap()

    calls_by_q = [[] for _ in range(NQ)]
    for call in sched:
        calls_by_q[call['q']].append(call)

    with tile.TileContext(nc) as tc:
        with tc.tile_pool(name="const", bufs=1) as cpool, \
             tc.tile_pool(name="work", bufs=10) as wpool, \
             tc.tile_pool(name="rh", bufs=8) as rhpool, \
             tc.tile_pool(name="oh", bufs=8) as ohpool, \
             tc.tile_pool(name="post", bufs=4) as postpool, \
             tc.tile_pool(name="qp", bufs=2, space="PSUM") as qpp, \
             tc.tile_pool(name="relp", bufs=3, space="PSUM") as relpp, \
             tc.tile_pool(name="tpp", bufs=2, space="PSUM") as tpp:

            idx_sb = cpool.tile([P, NCALL], mybir.dt.int32)
            loc_sb = cpool.tile([P, NTT], mybir.dt.float32)
            recip_sb = cpool.tile([P, NB], mybir.dt.float32)
            rel48_sb = cpool.tile([R2, d], mybir.dt.bfloat16)
            nc.sync.dma_start(out=idx_sb[:], in_=idx_d[:])
            nc.sync.dma_start(out=loc_sb[:], in_=loc_d[:])
            nc.sync.dma_start(out=recip_sb[:], in_=recip_d[:])
            nc.sync.dma_start(out=rel48_sb[:], in_=rel48_d[:])

            iota_i = cpool.tile([P, QE], mybir.dt.int32)
            nc.gpsimd.iota(iota_i[:], pattern=[[1, QE]], base=0,
                           channel_multiplier=0)
            iota_f = cpool.tile([P, QE], mybir.dt.float32)
            nc.vector.tensor_copy(out=iota_f[:], in_=iota_i[:])
            ident = cpool.tile([P, P], mybir.dt.float32)
            make_identity(nc, ident[:])

            def tile_pipe(gslice, st, qps):
                tt_ = st['tt']
                off, span = st['off'], st['span']
                rh = rhpool.tile([R2, P], mybir.dt.bfloat16, tag="rh")
                nc.sync.dma_start(out=rh[:], in_=relhot_d[tt_])
                relps = relpp.tile([P, d], mybir.dt.float32, space="PSUM",
                                   tag="relp")
                nc.tensor.matmul(out=relps[:], lhsT=rh[:], rhs=rel48_sb[:],
                                 start=True, stop=True)
                msg = wpool.tile([P, d], mybir.dt.float32, tag="msg")
                nc.vector.tensor_tensor(out=msg[:], in0=gslice, in1=relps[:],
                                        op=mybir.AluOpType.mult)
                oh = ohpool.tile([P, span], mybir.dt.float32, tag="oh")
                nc.vector.tensor_scalar(
                    out=oh[:], in0=iota_f[:, :span],
                    scalar1=loc_sb[:, tt_:tt_ + 1], scalar2=None,
                    op0=mybir.AluOpType.is_equal)
                nc.tensor.matmul(out=qps[:, off:off + span], lhsT=msg[:],
                                 rhs=oh[:], start=st['first'], stop=st['last'])

            for _rep in range(rep):
                for q in range(NQ):
                    qps = qpp.tile([P, QE], mybir.dt.float32, space="PSUM",
                                   tag="quad")
                    for call in calls_by_q[q]:
                        ct = call['ct']
                        if call['kind'] == 'pair':
                            g2 = wpool.tile([P, 2 * d], mybir.dt.float32,
                                            tag="g2")
                            nc.gpsimd.indirect_dma_start(
                                out=g2[:], out_offset=None, in_=tbl2_wide[:],
                                in_offset=bass.IndirectOffsetOnAxis(
                                    ap=idx_sb[:, ct:ct + 1], axis=0))
                            tile_pipe(g2[:, 0:d], call['tiles'][0], qps)
                            tile_pipe(g2[:, d:2 * d], call['tiles'][1], qps)
                        else:
                            g1 = wpool.tile([P, d], mybir.dt.float32, tag="g1")
                            nc.gpsimd.indirect_dma_start(
                                out=g1[:], out_offset=None, in_=tbl2[:],
                                in_offset=bass.IndirectOffsetOnAxis(
                                    ap=idx_sb[:, ct:ct + 1], axis=0))
                            tile_pipe(g1[:], call['tiles'][0], qps)
                    qsb = postpool.tile([P, QE], mybir.dt.float32, tag="qsb")
                    nc.scalar.copy(out=qsb[:], in_=qps[:])
                    for b4 in range(QB):
                        b = q * QB + b4
                        if b >= NB:
                            break
                        tps = tpp.tile([P, P], mybir.dt.float32, space="PSUM",
                                       tag="tp")
                        nc.tensor.transpose(out=tps[:],
                                            in_=qsb[:, b4 * P:(b4 + 1) * P],
                                            identity=ident[:])
                        osb = postpool.tile([P, d], mybir.dt.float32,
                                            tag="osb")
                        nc.vector.tensor_scalar(
                            out=osb[:], in0=tps[:],
                            scalar1=recip_sb[:, b:b + 1], scalar2=None,
                            op0=mybir.AluOpType.mult)
                        nc.sync.dma_start(out=out_d[b * P:(b + 1) * P, :],
                                          in_=osb[:])
    nc.compile()
    return nc


def make_tbl2(ego, sigma_k):
    # [N, 2, d] -> [2N, d]: row 2i = ego[i], row 2i+1 = ego[sigma[i]]
    n, d = ego.shape
    out = np.empty((n, 2, d), np.float32)
    out[:, 0, :] = ego
    out[:, 1, :] = ego[sigma_k]
    return out.reshape(2 * n, d)


def _assemble_output(pp, results, n_entities):
    parts = [results[k]["out"][:pp['blocks_per_core'][k] * P]
             for k in range(N_CORES)]
    return np.concatenate(parts, axis=0)[:n_entities]


_CACHE = {}


def _get_program(pp, n_rows):
    key = (pp['NTT'], pp['NCALL'],
           tuple((c['kind'], tuple((t['off'], t['span']) for t in c['tiles']))
                 for c in pp['sched']), n_rows)
    if key not in _CACHE:
        _CACHE[key] = build_program2(pp, n_rows)
    return _CACHE[key]


def kernel(ego_embed, edge_index, edge_type, relation_weight):
    from concourse.bass_utils import run_bass_kernel_spmd
    ego = np.asarray(ego_embed, np.float32)
    n, d = ego.shape
    r = np.asarray(relation_weight, np.float32).shape[0]
    pp = preprocess2(edge_index, edge_type, relation_weight, n, r, d)
    nc = _get_program(pp, n)
    in_maps = [{"tbl2": make_tbl2(ego, pp['sigma'][k]), "idx": pp['idx'][k],
                "loc": pp['loc'][k], "relhot": pp['relhot'][k],
                "recip": pp['recip'][k], "rel48": pp['rel48']}
               for k in range(N_CORES)]
    res = run_bass_kernel_spmd(nc, in_maps, list(range(N_CORES))).results
    return _assemble_output(pp, res, n).astype(np.float32)

